# revision 13
# baseline (speedup 1.0000x reference)
"""Trainium2 Bass kernel v3 for nn_AlgebraicAttention (relu-attention with
clipped relative-position bias).

Sharding: 8 cores = 4 batches x 2 head-groups. Core c: batch c//2, heads
(c%2)*8..+8. Attention fully local per core; two pairwise ReduceScatters
(token halves) write directly into the bf16 output.

v3: p-state-driven restructure. TRN2's PE runs at 2.4GHz only after ~3us of
continuous execution and drops to ~1.2GHz after any idle gap, so the whole
schedule is organized to never let the PE starve:
  - software pipelining ACROSS loop iterations: iteration i+1's V projection
    and Q(first-half) projection run as filler work inside iteration i's
    second attention phase; K(first-half) interleaves with the trailing
    O-tiles at the iteration boundary
  - all projection/O work is split into ~0.9us units and woven between
    attention score blocks so every dependent matmul has ready work ahead
  - input DMAs ride the SP queue, output DMAs + collectives the Pool queue,
    so next-iteration loads are never stuck behind this iteration's stores
  - static tile pools for the whole unrolled program (no per-iteration
    drains); double-buffered VA/G/consts allow cross-iteration overlap
"""

import sys

if "/opt/trn_rl_repo" not in sys.path:
    sys.path.insert(0, "/opt/trn_rl_repo")

import numpy as np
import ml_dtypes

_bf16 = ml_dtypes.bfloat16

B, T, D = 4, 1024, 1024
H, DH = 16, 64
MAXREL = 32
NCORES = 8
HL = 8          # heads per core
DL = HL * DH    # 512 local dims
SCALE = DH ** -0.5
NEG = -1.0e30
EPS = 1.0e-6

_cached = {}
DIAG_NO_DVE = False
DIAG_NO_ATTN = False
DIAG_PROJ_ONLY = False
PIPE_N = 3
PSF_N = 3
NUM_LAG = 8


def _tf32_round(a: np.ndarray) -> np.ndarray:
    """TF32-round f32 host data (matches PE f32r input rounding)."""
    u = np.ascontiguousarray(a, dtype=np.float32).view(np.uint32).copy()
    u += np.uint32(0xFFF) + ((u >> np.uint32(13)) & np.uint32(1))
    u &= np.uint32(0xFFFFE000)
    return u.view(np.float32)


def _register_custom_dve():
    """relu(S + G) + 1e-6*(G >= -1e20): fused bias-add + causal mask + relu +
    epsilon for the attention epilogue, one DVE instruction per block."""
    import numpy as np
    import concourse.dve_ops as dops
    from concourse.dve_spec import Spec, Src0, Src1, C0, C1, relu, lower
    from concourse.dve_uop import DveOpSpec

    name = "RELU_BIAS_EPS_ANT"
    for op in dops.OPS:
        if op.name == name:
            return op
    body = relu(Src0 + Src1) + (Src1 >= C0) * C1

    def ref(in0, in1, s0, s1, imm2):
        in0 = in0.astype(np.float32)
        in1 = in1.astype(np.float32)
        return (np.maximum(in0 + in1, 0.0) + (in1 >= s0) * s1).astype(np.float32)

    spec = Spec(body=body, reference=ref)
    shas = {}
    for ver in ("v3", "v4"):
        ds = DveOpSpec(name=name, opcode=0, uops=lower(spec, ver=ver), rd1_en=True)
        shas[ver] = ds.sha(ver)
    op = dops.DveOp(name, spec, subdim=False, uops_sha=shas)
    dops.OPS.append(op)
    dops._SUB_OPCODE_FOR_NAME[name] = dops._CUSTOM_DVE_ROW_BASE + len(dops.OPS) - 1
    assert dops._SUB_OPCODE_FOR_NAME[name] < 0x20
    return op


def _build_nc(sim_mode: bool = False, loop_n: int = 1):
    import concourse.bacc as bacc
    import concourse.mybir as mybir
    import concourse.tile as tile

    CUSTOM_OP = _register_custom_dve()

    f32 = mybir.dt.float32
    bf16 = mybir.dt.bfloat16
    f32r = mybir.dt.float32r

    nc = bacc.Bacc("TRN2", target_bir_lowering=False)

    # ---- kernel I/O (per-core shapes) ----
    xT_d = nc.dram_tensor("xT", [128, 8 * T], f32r, kind="ExternalInput")
    wqT_d = nc.dram_tensor("wqT", [128, 8 * DL], f32r, kind="ExternalInput")
    wkT_d = nc.dram_tensor("wkT", [128, 8 * DL], f32r, kind="ExternalInput")
    wv_d = nc.dram_tensor("wv", [128, 8 * DL], f32r, kind="ExternalInput")
    woT_d = nc.dram_tensor("woT", [128, 4 * D], bf16, kind="ExternalInput")
    bqk_d = nc.dram_tensor("bqk", [128, 9], f32, kind="ExternalInput")
    bv_d = nc.dram_tensor("bv_row", [1, DL], f32r, kind="ExternalInput")
    bo_d = nc.dram_tensor("bo_row", [1, D], bf16, kind="ExternalInput")
    ones_d = nc.dram_tensor("ones_row", [1, 128], f32r, kind="ExternalInput")
    onesb_d = nc.dram_tensor("ones_b", [1, 128], bf16, kind="ExternalInput")
    G_d = nc.dram_tensor("G", [128, HL * 1024], bf16, kind="ExternalInput")
    epsG_d = nc.dram_tensor("epsG", [128, 1024], bf16, kind="ExternalInput")
    rel0_d = nc.dram_tensor("rel0", [128, HL], f32, kind="ExternalInput")
    out_d = nc.dram_tensor("out", [T // 2, D], bf16, kind="ExternalOutput")

    RELU = mybir.ActivationFunctionType.Relu
    IDENT = mybir.ActivationFunctionType.Identity
    COPY = mybir.ActivationFunctionType.Copy
    ADD = mybir.AluOpType.add
    MULT = mybir.AluOpType.mult

    with tile.TileContext(nc) as tc:
        with (
            tc.tile_pool(name="pp", bufs=1) as pp,
            tc.tile_pool(name="pp2", bufs=2) as pp2,
            tc.tile_pool(name="sp", bufs=2) as sp,
            tc.tile_pool(name="psS", bufs=PIPE_N, space="PSUM") as psS,
            tc.tile_pool(name="psN", bufs=2, space="PSUM") as psN,
            tc.tile_pool(name="psF", bufs=PSF_N, space="PSUM") as psF,
            tc.tile_pool(name="dram", bufs=1, space="DRAM") as dr,
        ):

            class It:
                """Per-iteration tiles + unit closures."""

                def __init__(s, it):
                    s.it = it
                    # persistent (bufs=1) tiles: same buffer each iteration
                    s.QT = [pp.tile([128, T], f32r, tag=f"QT{i}", name=f"QT{i}")
                            for i in range(4)]
                    s.KT = [pp.tile([128, T], f32r, tag=f"KT{i}", name=f"KT{i}")
                            for i in range(4)]
                    s.attnT = [pp.tile([128, T], bf16, tag=f"attnT{i}",
                                       name=f"attnT{i}") for i in range(4)]
                    s.xA = pp.tile([128, 8 * 512], f32r, tag="xA", name="xA")
                    s.xB = pp.tile([128, 8 * 512], f32r, tag="xB", name="xB")
                    s.wq = pp.tile([128, 8 * DL], f32r, tag="wq", name="wq")
                    s.wk = pp.tile([128, 8 * DL], f32r, tag="wk", name="wk")
                    s.wv = pp.tile([128, 8 * DL], f32r, tag="wv", name="wv")
                    s.wo = pp.tile([128, 4 * D], bf16, tag="wo", name="wo")
                    s.epsG = pp.tile([128, 1024], bf16, tag="epsG", name="epsG")
                    s.Gall = pp.tile([128, HL * 1024], bf16, tag="Gall",
                                     name="Gall")
                    # double-buffered tiles (read through W4 while the
                    # successor iteration's writes are already emitted)
                    s.VA = [pp2.tile([128, HL * 65], bf16, tag=f"VA{i}",
                                     name=f"VA{i}") for i in range(8)]
                    s.G = [s.Gall[:, i * 1024:(i + 1) * 1024] for i in range(HL)]
                    s.bqk = pp2.tile([128, 9], f32, tag="bqk", name="bqk")
                    s.bv = pp2.tile([1, DL], f32r, tag="bv", name="bv")
                    s.bo = pp2.tile([1, D], bf16, tag="bo", name="bo")
                    s.ones = pp2.tile([1, 128], f32r, tag="ones", name="ones")
                    s.onesb = pp2.tile([1, 128], bf16, tag="onesb", name="onesb")
                    s.rel0 = pp2.tile([128, HL], f32, tag="rel0", name="rel0")
                    s.cc_a = dr.tile([T // 2, D], bf16, tag="cc_a", name="cc_a",
                                     space="DRAM")
                    s.cc_b = dr.tile([T // 2, D], bf16, tag="cc_b", name="cc_b",
                                     space="DRAM")
                    s.cco_a = dr.tile([T // 4, D], bf16, tag="cco_a",
                                      name="cco_a", space="DRAM")
                    s.cco_b = dr.tile([T // 4, D], bf16, tag="cco_b",
                                      name="cco_b", space="DRAM")
                    s.wqv = [s.wq[:, i * DL:(i + 1) * DL] for i in range(8)]
                    s.wkv = [s.wk[:, i * DL:(i + 1) * DL] for i in range(8)]
                    s.wvv = [s.wv[:, i * DL:(i + 1) * DL] for i in range(8)]
                    s.wov = [s.wo[:, i * D:(i + 1) * D] for i in range(4)]

                def x_slice(s, ct, c0, c1):
                    buf, base = (s.xA, 0) if c1 <= 512 else (s.xB, 512)
                    return buf[:, ct * 512 + c0 - base:ct * 512 + c1 - base]

                # ---- DMA loads (SP queue) ----
                def loads_first(s):
                    nc.sync.dma_start(s.wv[:], wv_d[:])

                def loads_main_early(s):
                    # xA/wq WAR clears at phase C end (readers ran in W4/C of
                    # the previous iteration)
                    nc.sync.dma_start(s.xA[:], xT_d[:, 0:4096])
                    nc.sync.dma_start(s.bqk[:], bqk_d[:])
                    nc.sync.dma_start(s.bv[:], bv_d[:])
                    nc.sync.dma_start(s.bo[:], bo_d[:])
                    nc.sync.dma_start(s.ones[:], ones_d[:])
                    nc.sync.dma_start(s.onesb[:], onesb_d[:])
                    nc.sync.dma_start(s.rel0[:], rel0_d[:])
                    nc.sync.dma_start(s.wq[:], wqT_d[:])

                def loads_main_late(s):
                    # xB/wk read by this iteration's W3 fillers
                    nc.sync.dma_start(s.xB[:], xT_d[:, 4096:8192])
                    nc.sync.dma_start(s.wk[:], wkT_d[:])

                def loads_late(s):
                    # emitted after the previous iteration's last readers of
                    # these bufs=1 tiles (W4 customs / far blocks / o47)
                    nc.sync.dma_start(s.Gall[:, 0:HL * 512], G_d[:, 0:HL * 512])
                    nc.sync.dma_start(s.Gall[:, HL * 512:HL * 1024],
                                      G_d[:, HL * 512:HL * 1024])
                    nc.sync.dma_start(s.wo[:], woT_d[:])
                    nc.sync.dma_start(s.epsG[:], epsG_d[:])

                # ---- V projection units (2 per token tile) ----
                def v_units(s):
                    units = []
                    cell = {}

                    def halfA(tt):
                        ps = psF.tile([128, 512], f32, tag="fps", name="fps")
                        cell[tt] = ps
                        for ct in range(4):
                            nc.tensor.matmul(
                                ps[:], s.x_slice(ct, tt * 128, (tt + 1) * 128),
                                s.wvv[ct], start=(ct == 0), stop=False)

                    def halfB(tt):
                        ps = cell.pop(tt)
                        for ct in range(4, 8):
                            nc.tensor.matmul(
                                ps[:], s.x_slice(ct, tt * 128, (tt + 1) * 128),
                                s.wvv[ct], start=False, stop=False)
                        nc.tensor.matmul(ps[:], s.ones[:], s.bv[:],
                                         start=False, stop=True)
                        va = s.VA[tt]
                        out_ap = va[:, 0:520].rearrange(
                            "p (h c) -> p h c", c=65)[:, :, 0:64]
                        in_ap = ps[:, 0:512].rearrange(
                            "p (h c) -> p h c", c=64)
                        nc.scalar.activation(out_ap, in_ap, COPY)
                        ones_ap = va[:, 0:520].rearrange(
                            "p (h c) -> p h c", c=65)[:, :, 64:65]
                        nc.gpsimd.memset(ones_ap, 1.0)

                    for tt in range(8):
                        units.append(lambda tt=tt: halfA(tt))
                        units.append(lambda tt=tt: halfB(tt))
                    return units

                # ---- Q/K projection units (2 per (tb, db)) ----
                def qk_units(s, tb, which):
                    units = []
                    cell = {}
                    dst, w_sb, bcol = ((s.QT, s.wqv, 0) if which == "q"
                                       else (s.KT, s.wkv, 4))

                    def halfA(db):
                        ps = psF.tile([128, 512], f32, tag="fps", name="fps")
                        cell[db] = ps
                        for ct in range(4):
                            nc.tensor.matmul(
                                ps[:], w_sb[ct][:, db * 128:(db + 1) * 128],
                                s.x_slice(ct, tb * 512, (tb + 1) * 512),
                                start=(ct == 0), stop=False)

                    def halfB(db):
                        ps = cell.pop(db)
                        for ct in range(4, 8):
                            nc.tensor.matmul(
                                ps[:], w_sb[ct][:, db * 128:(db + 1) * 128],
                                s.x_slice(ct, tb * 512, (tb + 1) * 512),
                                start=False, stop=(ct == 7))
                        nc.scalar.activation(
                            dst[db][:, tb * 512:(tb + 1) * 512], ps[:], IDENT,
                            bias=s.bqk[:, bcol + db:bcol + db + 1])

                    for db in range(4):
                        units.append(lambda db=db: halfA(db))
                        units.append(lambda db=db: halfB(db))
                    return units

                # ---- O-projection units (2 per token tile) ----
                def o_units(s, tbs):
                    units = []
                    cell = {}

                    def half(tb, eb):
                        o_sb = cell.get(tb)
                        if o_sb is None:
                            o_sb = sp.tile([128, D], bf16, tag="o_sb",
                                           name="o_sb", bufs=2)
                            cell[tb] = o_sb
                        ps = psF.tile([128, 512], f32, tag="fps", name="fps")
                        for dt4 in range(4):
                            nc.tensor.matmul(
                                ps[:], s.attnT[dt4][:, tb * 128:(tb + 1) * 128],
                                s.wov[dt4][:, eb * 512:(eb + 1) * 512],
                                start=(dt4 == 0), stop=False)
                        # bo/2 per core; the pair-sum restores bo
                        nc.tensor.matmul(ps[:], s.onesb[:],
                                         s.bo[:, eb * 512:(eb + 1) * 512],
                                         start=False, stop=True)
                        nc.scalar.activation(
                            o_sb[:, eb * 512:(eb + 1) * 512], ps[:], COPY)
                        if eb == 1:
                            # SP queue: keeps the Pool queue (bcasts) snappy
                            cc = s.cc_a if tb < 4 else s.cc_b
                            nc.sync.dma_start(
                                cc[(tb % 4) * 128:(tb % 4 + 1) * 128, :],
                                o_sb[:])
                            cell.pop(tb)

                    for tb in tbs:
                        units.append(lambda tb=tb: half(tb, 0))
                        units.append(lambda tb=tb: half(tb, 1))
                    return units

                # ---- collectives (Pool queue) ----
                def rs1(s):
                    if sim_mode:
                        nc.gpsimd.dma_start(s.cco_a[:, :], s.cc_a[0:T // 4, :])
                    else:
                        nc.gpsimd.collective_compute(
                            "ReduceScatter", ADD,
                            replica_groups=[[0, 1], [2, 3], [4, 5], [6, 7]],
                            ins=[s.cc_a.opt()], outs=[s.cco_a.opt()],
                        )
                    nc.gpsimd.dma_start(out_d[0:T // 4, :], s.cco_a[:, :])

                def rs2(s):
                    if sim_mode:
                        nc.gpsimd.dma_start(s.cco_b[:, :], s.cc_b[0:T // 4, :])
                    else:
                        nc.gpsimd.collective_compute(
                            "ReduceScatter", ADD,
                            replica_groups=[[0, 1], [2, 3], [4, 5], [6, 7]],
                            ins=[s.cc_b.opt()], outs=[s.cco_b.opt()],
                        )
                    nc.gpsimd.dma_start(out_d[T // 4:T // 2, :], s.cco_b[:, :])

                # ---- attention phase (qb in {0, 1}), fillers woven in ----
                def attention(s, qb, fillers, npre=0):
                    if DIAG_NO_ATTN:
                        for f in fillers:
                            f()
                        return
                    nkb = 4 * (qb + 1)
                    per_hp = nkb * 2
                    blocks = [(hp, kb, par) for hp in range(4)
                              for kb in range(nkb) for par in (0, 1)]
                    nblocks = len(blocks)
                    nf = len(fillers)
                    emitted = [0]
                    done = [0]

                    def maybe_fill():
                        want = (done[0] * nf) // nblocks
                        while emitted[0] < want:
                            fillers[emitted[0]]()
                            emitted[0] += 1

                    for _ in range(min(npre, nf)):
                        fillers[emitted[0]]()
                        emitted[0] += 1

                    nums = {}
                    s_ps = {}
                    w_cell = {}

                    def emit_scores(hp, kb, par):
                        if kb == 0 and par == 0:
                            nums[hp] = {
                                0: psN.tile([65, 512], f32, tag="num",
                                            name="num0"),
                                1: psN.tile([65, 512], f32, tag="num",
                                            name="num1")}
                        prow = par * 64
                        ps = psS.tile([128, 512], f32, tag="s", name="s")
                        s_ps[(hp, kb, par)] = ps
                        off = kb * 128 - qb * 512
                        lo = max(0, off)
                        nc.tensor.matmul(
                            ps[:, 0:512 - lo],
                            s.KT[hp][prow:prow + 64, kb * 128:(kb + 1) * 128],
                            s.QT[hp][prow:prow + 64,
                                     qb * 512 + lo:(qb + 1) * 512],
                            start=True, stop=True,
                            tile_position=(prow, 0),
                        )

                    def emit_epi(hp, kb, par):
                        # relu epilogue (DVE near / ACT far); the matching num
                        # matmul is emitted NUM_LAG blocks later so its w_sb
                        # is long-ready when it reaches the PE queue head
                        hl = 2 * hp + par
                        ps = s_ps.pop((hp, kb, par))
                        off = kb * 128 - qb * 512
                        lo = max(0, off)
                        w_sb = sp.tile([128, 512], bf16,
                                       tag="w", name="w_sb", bufs=12)
                        w_cell[(hp, kb, par)] = w_sb
                        if off > -256:
                            nc.vector._custom_dve(
                                CUSTOM_OP, out=w_sb[:, 0:512 - lo],
                                in0=ps[:, 0:512 - lo],
                                in1=s.G[hl][:, 384 - off + lo:896 - off],
                                s0=-1.0e20, s1=EPS)
                        else:
                            nc.scalar.activation(
                                w_sb[:], ps[:], RELU,
                                bias=s.rel0[:, hl:hl + 1])

                    def emit_num(hp, kb, par):
                        hl = 2 * hp + par
                        off = kb * 128 - qb * 512
                        lo = max(0, off)
                        w_sb = w_cell.pop((hp, kb, par))
                        va = s.VA[kb][:, hl * 65:(hl + 1) * 65]
                        num = nums[hp]
                        if off > -256:
                            nc.tensor.matmul(num[par][:, lo:512], va,
                                             w_sb[:, 0:512 - lo],
                                             start=(kb == 0),
                                             stop=(kb == nkb - 1))
                        else:
                            nc.tensor.matmul(num[par][:], va, w_sb[:],
                                             start=(kb == 0), stop=False)
                            nc.tensor.matmul(num[par][:], va,
                                             s.epsG[:, 512:1024],
                                             start=False, stop=False)

                    def emit_norm(hp):
                        num = nums.pop(hp)
                        for par in (0, 1):
                            prow = par * 64
                            den = sp.tile([1, 512], f32, tag="den", name="den")
                            nc.scalar.activation(
                                den[:], num[par][64:65, :], IDENT,
                                bias=s.bqk[0:1, 8:9])
                            r1 = sp.tile([1, 512], f32, tag="r1", name="r1")
                            nc.vector.reciprocal(r1[:], den[:])
                            rb = sp.tile([64, 512], f32, tag="rb", name="rb")
                            nc.gpsimd.partition_broadcast(rb[:], r1[:])
                            nc.vector.tensor_tensor(
                                s.attnT[hp][prow:prow + 64,
                                            qb * 512:(qb + 1) * 512],
                                num[par][0:64, :], rb[:], MULT)

                    PIPE = PIPE_N
                    LAG = NUM_LAG
                    for i in range(min(PIPE, nblocks)):
                        emit_scores(*blocks[i])
                    for i in range(nblocks):
                        if i + PIPE < nblocks:
                            emit_scores(*blocks[i + PIPE])
                        emit_epi(*blocks[i])
                        j = i - LAG
                        if j >= 0:
                            emit_num(*blocks[j])
                            if j % per_hp == per_hp - 1:
                                emit_norm(blocks[j][0])
                        done[0] += 1
                        maybe_fill()
                    for j in range(max(0, nblocks - LAG), nblocks):
                        emit_num(*blocks[j])
                        if j % per_hp == per_hp - 1:
                            emit_norm(blocks[j][0])
                    while emitted[0] < len(fillers):
                        fillers[emitted[0]]()
                        emitted[0] += 1

            def interleave(a, b):
                out = []
                ia = ib = 0
                for _ in range(max(len(a), len(b))):
                    if ia < len(a):
                        out.append(a[ia]); ia += 1
                    if ib < len(b):
                        out.append(b[ib]); ib += 1
                return out

            # ---------- software-pipelined emission ----------
            cur = It(0)
            cur.loads_first()
            cur.loads_main_early()
            cur.loads_main_late()
            cur.loads_late()
            for u in cur.v_units():
                u()
            for u in interleave(cur.qk_units(0, "q"), cur.qk_units(0, "k")):
                u()
            prev = None
            for it in range(loop_n):
                if prev is not None:
                    # phase C: trailing O-tiles of prev + K(tb0) of cur + RS2
                    ou = ([] if DIAG_PROJ_ONLY
                          else prev.o_units([4, 5, 6, 7]))
                    for u in interleave(ou, cur.qk_units(0, "k")):
                        u()
                    if not DIAG_PROJ_ONLY:
                        prev.rs2()
                    cur.loads_late()
                # W3: attention qb=0, fillers = Q/K tb=1 projections; a few
                # prefix units cover the G reload latency
                nxt = It(it + 1) if it + 1 < loop_n else None
                if nxt is not None:
                    nxt.loads_first()
                    nxt.loads_main_early()
                cur.attention(0, interleave(cur.qk_units(1, "q"),
                                            cur.qk_units(1, "k")), npre=4)
                if nxt is not None:
                    nxt.loads_main_late()
                # W4: attention qb=1, fillers = O-tiles 0-3 + next iteration's
                # V and Q(tb0) projections; RS1 placed late so its cc_a wait
                # never blocks the Pool queue's broadcasts
                fillers = ([] if DIAG_PROJ_ONLY
                           else cur.o_units([0, 1, 2, 3]))
                if nxt is not None:
                    fillers += nxt.v_units()
                    if not DIAG_PROJ_ONLY:
                        fillers += [cur.rs1]
                    fillers += nxt.qk_units(0, "q")
                elif not DIAG_PROJ_ONLY:
                    fillers += [cur.rs1]
                cur.attention(1, fillers)
                prev, cur = cur, nxt
            # final phase C
            if not DIAG_PROJ_ONLY:
                for u in prev.o_units([4, 5, 6, 7]):
                    u()
                prev.rs2()

    nc.compile()
    return nc


def _host_inputs(x, mask, Wq, bq, Wk, bk, Wv, bv, Wo, bo, rel_bias, n_head):
    """Build the 8 per-core input maps."""
    x = np.asarray(x, np.float32)
    Wq, bq = np.asarray(Wq, np.float32), np.asarray(bq, np.float32)
    Wk, bk = np.asarray(Wk, np.float32), np.asarray(bk, np.float32)
    Wv, bv = np.asarray(Wv, np.float32), np.asarray(bv, np.float32)
    Wo, bo = np.asarray(Wo, np.float32), np.asarray(bo, np.float32)
    rel = np.asarray(rel_bias, np.float32)

    bo_row = (bo[None, :] * 0.5).astype(_bf16)  # halved; pair-sum restores bo
    ones_row = _tf32_round(np.ones((1, 128), np.float32))
    kk = np.arange(128)[:, None]
    wprime = np.arange(1024)[None, :]
    dmat = kk + 384 - wprime                     # d = k_abs - q_abs
    epsG = np.where(dmat <= 0, EPS, 0.0).astype(np.float32).astype(_bf16)

    in_maps = []
    for c in range(NCORES):
        b, g = divmod(c, 2)
        sl = slice(g * DL, (g + 1) * DL)
        xT = _tf32_round(x[b].T.copy())
        wqT = _tf32_round((Wq[sl, :] * SCALE).T.copy())
        wkT = _tf32_round(Wk[sl, :].T.copy())
        wvT = _tf32_round(Wv[sl, :].T.copy())
        woT = np.ascontiguousarray(Wo[:, sl].T, np.float32).astype(_bf16)
        bv_row = _tf32_round(bv[sl][None, :].copy())
        bqk = np.zeros((128, 9), np.float32)
        bqk[:, 8] = EPS
        for db in range(4):
            bqk[:, db] = (bq * SCALE)[sl][db * 128:(db + 1) * 128]
            bqk[:, 4 + db] = bk[sl][db * 128:(db + 1) * 128]
        heads = np.arange(g * HL, (g + 1) * HL)
        G = np.empty((HL, 128, 1024), np.float32)
        for i, h in enumerate(heads):
            vals = rel[np.clip(dmat, -(MAXREL - 1), MAXREL - 1) + MAXREL - 1, h]
            G[i] = np.where(dmat <= 0, vals, NEG)
        rel0 = np.tile(rel[0, heads][None, :], (128, 1)).astype(np.float32)

        def blocked(a, nb):
            # [nb*128, X] -> [128, nb*X] ct-major for single coalesced DMA
            return np.ascontiguousarray(
                a.reshape(nb, 128, a.shape[1]).transpose(1, 0, 2)
                .reshape(128, nb * a.shape[1]))
        xTh = xT.reshape(8, 128, 2, 512)
        xT2 = np.concatenate(
            [np.ascontiguousarray(xTh[:, :, h].transpose(1, 0, 2)
                                  .reshape(128, 4096)) for h in (0, 1)], axis=1)
        in_maps.append({
            "xT": xT2, "wqT": blocked(wqT, 8),
            "wkT": blocked(wkT, 8), "wv": blocked(wvT, 8),
            "woT": blocked(woT, 4), "bqk": bqk, "bv_row": bv_row,
            "bo_row": bo_row, "ones_row": ones_row,
            "ones_b": ones_row.astype(_bf16),
            "G": np.ascontiguousarray(
                G.astype(_bf16).transpose(1, 0, 2).reshape(128, HL * 1024)),
            "epsG": epsG, "rel0": rel0,
        })
    return in_maps


def _assemble(results):
    out = np.empty((B, T, D), np.float32)
    for c in range(NCORES):
        b, g = divmod(c, 2)
        r = np.asarray(results[c]["out"], dtype=np.float32)
        # RS#1 shard: tokens [g*256, g*256+256); RS#2: [512+g*256, +256)
        out[b, g * 256:(g + 1) * 256, :] = r[0:256]
        out[b, 512 + g * 256:512 + (g + 1) * 256, :] = r[256:512]
    return out


def get_nc():
    if "nc" not in _cached:
        _cached["nc"] = _build_nc()
    return _cached["nc"]


def kernel(**inputs) -> np.ndarray:
    import time
    from concourse.bass_utils import run_bass_kernel_spmd

    in_maps = _host_inputs(**inputs)
    nc = get_nc()
    res = None
    for attempt in range(2):
        try:
            res = run_bass_kernel_spmd(nc, in_maps,
                                       core_ids=list(range(NCORES)))
            break
        except Exception:
            # transient tunnel/device desync seen on cold first executions;
            # one retry recovers (observed empirically)
            if attempt:
                raise
            time.sleep(2.0)
    return _assemble(res.results)
